# revision 2
# baseline (speedup 1.0000x reference)
"""Multi-head attention (RoPE, causal) on 8 TRN2 NeuronCores.

Sharding: core c -> batch b = c//2, head-group g = c%2 (8 of 16 heads).

v6 design (on top of v5):
- ALL inputs fused into one fp16 dram tensor BLOB [128, 9, 8, 512]:
  segments 0-3 = XT query chunks, 4 = WQ, 5 = WK, 6 = WV, 7 = OC,
  8 = CC(2048 cols) + SN(2048 cols) as fp16. One input handle per core
  cuts the per-dispatch host/PJRT overhead (measured ~0.5 ms for 8
  handles vs 1) and host->device transfer bookkeeping.
- kernel() dispatches through a cached jit/shard_map callable built
  once (mirrors concourse.bass2jax.run_bass_via_pjrt) instead of
  re-entering run_bass_kernel_spmd each call.

v5 design:
- all 16-bit tensors fp16 (PSUM accumulation stays f32); everything
  SBUF-resident, no DRAM staging roundtrips.
- q/k stored per head-PAIR: head A dims on partitions 0-63, head B on
  64-127, natural dim order (rope pairs adjacent). The rope cross-term
  partition swap is a stream_shuffle with mask i^1; the sin sign pattern
  is baked into SN host-side. Scores for the two heads issue as two K=64
  matmuls with tile_position (0,0)/(64,0) into adjacent PSUM banks ->
  concurrent on HW.
- exp reads the pair's [128, 1024] PSUM span in ONE activation.
- V' carries 64 replicated ones-columns (M=128): pO rows 64-127 hold the
  softmax denominator replicated 64x, so normalize is reciprocal+mul
  with no partition broadcast. Matmul cost is free-dim based, so the
  extra 63 columns are free.
- query chunks of 512: causal work only; o_proj per chunk overlaps the
  last pair's attention.
"""

import numpy as np
from contextlib import ExitStack

import ml_dtypes

import jax
from jax.sharding import Mesh, NamedSharding, PartitionSpec

try:
    from jax.experimental.shard_map import shard_map
except ImportError:
    from jax.sharding import shard_map

import concourse.bacc as bacc
import concourse.bass as bass
import concourse.mybir as mybir
import concourse.tile as tile
from concourse.bass2jax import (
    _bass_exec_p,
    install_neuronx_cc_hook,
    partition_id_tensor,
)
from concourse.masks import make_upper_triangular

F32 = mybir.dt.float32
F32R = mybir.dt.float32r
F16 = mybir.dt.float16
AF = mybir.ActivationFunctionType

D = 1024
S = 2048
NH = 16
DK = 64
HPC = 8          # heads per core
HD = HPC * DK    # 512
NCORES = 8
THETA = 10000.0

NK = D // 128    # 8 x-dim k-tiles
NS = S // 128    # 16 key tiles
NC_ = 4          # query chunks of 512
NP = 4           # head pairs per core

# BLOB segments (each [128, 8, 512] fp16 = 4096 cols)
SEG_XT0 = 0      # ..3: XT query chunks
SEG_WQ = 4
SEG_WK = 5
SEG_WV = 6
SEG_OC = 7
SEG_CS = 8       # CC cols 0:2048 (sub 0-3), SN cols 2048:4096 (sub 4-7)

_CACHE = {}

_SWAP_MASK = [i ^ 1 for i in range(32)]


def _copy(nc, use_scalar, out, in_):
    if use_scalar:
        nc.scalar.copy(out, in_)
    else:
        nc.vector.tensor_copy(out, in_)


def _build_nc():
    nc = bacc.Bacc(None, target_bir_lowering=False)

    BLOB = nc.dram_tensor("BLOB", [128, 9, NK, 512], F16, kind="ExternalInput")
    OT = nc.dram_tensor("OT", [D, S], F16, kind="ExternalOutput")

    with tile.TileContext(nc) as tc, ExitStack() as ctx:
        const = ctx.enter_context(tc.tile_pool(name="const", bufs=1))
        resv = ctx.enter_context(tc.tile_pool(name="resv", bufs=1))
        psp = ctx.enter_context(tc.tile_pool(name="psp", bufs=2,
                                             space="PSUM"))
        rp = ctx.enter_context(tc.tile_pool(name="rp", bufs=2))
        ptp = ctx.enter_context(tc.tile_pool(name="ptp", bufs=4))
        rp2 = ctx.enter_context(tc.tile_pool(name="rp2", bufs=2))

        # constants
        tri32 = const.tile([128, 128], F32, tag="tri32")
        make_upper_triangular(nc, tri32, val=1.0, diag=True)
        tri = const.tile([128, 128], F16, tag="tri")
        nc.vector.tensor_copy(tri, tri32)
        # CC/SN arrive fp16 inside the blob; rope math wants f32 rows
        ccsn_sb = const.tile([128, NK, 512], F16, tag="ccsn")
        cc_sb = const.tile([128, S], F32, tag="cc")
        sn_sb = const.tile([128, S], F32, tag="sn")

        # resident tensors
        xtc = [resv.tile([128, NK, 512], F16, tag=f"xtc{c}",
                         name=f"xtc{c}") for c in range(NC_)]
        vp = resv.tile([128, NS, HPC * 128], F16, tag="vp")
        wv_sb = resv.tile([128, NK, HD], F16, tag="wv")
        wpp = ctx.enter_context(tc.tile_pool(name="wpp", bufs=2))
        wq_pair, wk_pair = {}, {}

        def load_qk_pair(pn):
            wq_pair[pn] = wpp.tile([128, NK, 128], F16, tag="wqp",
                                   name=f"wq{pn}")
            nc.scalar.dma_start(out=wq_pair[pn],
                                in_=BLOB[:, SEG_WQ, :,
                                         pn * 128:(pn + 1) * 128])
            wk_pair[pn] = wpp.tile([128, NK, 128], F16, tag="wkp",
                                   name=f"wk{pn}")
            nc.sync.dma_start(out=wk_pair[pn],
                              in_=BLOB[:, SEG_WK, :,
                                       pn * 128:(pn + 1) * 128])
        oc_sb = resv.tile([128, NK, 512], F16, tag="oc")
        oc_fl = oc_sb.rearrange("p a b -> p (a b)")
        qp = [resv.tile([128, S], F16, tag=f"qp{p}", name=f"qp{p}")
              for p in range(NP)]
        kp = [resv.tile([128, S], F16, tag=f"kp{p}", name=f"kp{p}")
              for p in range(NP)]
        ypr = [resv.tile([128, S], F16, tag=f"y{p}", name=f"y{p}")
               for p in range(NP)]

        # input DMAs: xtall by s-chunk so chunk-0 projections start early;
        # spread loads over the three DMA-capable queues by criticality
        def xt_chunk(eng, cd):
            eng.dma_start(out=xtc[cd], in_=BLOB[:, SEG_XT0 + cd])

        xt_chunk(nc.sync, 0)
        load_qk_pair(0)
        nc.gpsimd.dma_start(out=ccsn_sb, in_=BLOB[:, SEG_CS])
        ccsn_fl = ccsn_sb.rearrange("p a b -> p (a b)")
        nc.vector.tensor_copy(cc_sb, ccsn_fl[:, 0:S])
        nc.vector.tensor_copy(sn_sb, ccsn_fl[:, S:2 * S])
        xt_chunk(nc.gpsimd, 1)
        xt_chunk(nc.sync, 2)
        xt_chunk(nc.scalar, 3)
        load_qk_pair(1)
        nc.sync.dma_start(out=wv_sb, in_=BLOB[:, SEG_WV])
        nc.scalar.dma_start(out=oc_sb, in_=BLOB[:, SEG_OC])

        def emit_proj_chunk(wsb, p, c, dest):
            """q/k projection + rope for pair p, query chunk c."""
            cs = slice(c * 512, (c + 1) * 512)
            ps = psp.tile([128, 512], F32, tag="ps", name="ps_proj")
            for k in range(NK):
                nc.tensor.matmul(ps, wsb[:, k, :], xtc[c][:, k, :],
                                 start=(k == 0), stop=(k == NK - 1))
            u = rp.tile([128, 512], F32, tag="U", name="u")
            nc.vector.tensor_mul(u, ps, cc_sb[:, cs])
            w2 = rp.tile([128, 512], F32, tag="W2", name="w2")
            nc.vector.tensor_mul(w2, ps, sn_sb[:, cs])
            # rope combine: out[2i] = e*c - o*s ; out[2i+1] = o*c + e*s
            # (sign baked into SN; swap adjacent partitions then add).
            # f32 temporaries: only the final add rounds to fp16.
            w2s = rp.tile([128, 512], F32, tag="W2s", name="w2s")
            nc.vector.stream_shuffle(w2s, w2, _SWAP_MASK)
            nc.gpsimd.tensor_add(dest[:, cs], u, w2s)

        def emit_v_tile(j):
            # ones columns of V' (denominator trick). Head A blocks are
            # [dims | ones]; head B blocks are [ones | dims] so B's
            # numerator lands on partitions 64-127 (same-start muls).
            # first tiles' memsets on DVE: the gpsimd queue is still
            # draining input DMAs when attention p0 c0 needs them
            meng = nc.vector if j < 4 else nc.gpsimd
            vj0 = vp[:, j, :].rearrange("p (q two e) -> p q two e",
                                        two=2, e=128)
            meng.memset(vj0[:, :, 0, DK:128], 1.0)
            meng.memset(vj0[:, :, 1, 0:DK], 1.0)
            psv = psp.tile([128, 512], F32, tag="ps", name="psv")
            for k in range(NK):
                nc.tensor.matmul(psv,
                                 xtc[j // 4][:, k, (j % 4) * 128:(j % 4 + 1) * 128],
                                 wv_sb[:, k, :],
                                 start=(k == 0), stop=(k == NK - 1))
            vj = vp[:, j, :].rearrange("p (q two e) -> p q two e",
                                       two=2, e=128)
            sv = psv.rearrange("p (q two e) -> p q two e", two=2, e=DK)
            _copy(nc, j % 2 == 0, vj[:, :, 0, 0:DK], sv[:, :, 0, :])
            _copy(nc, j % 2 == 1, vj[:, :, 1, DK:128], sv[:, :, 1, :])

        def emit_attn_chunk(p, c):
            cs = slice(c * 512, (c + 1) * 512)
            jmax = 4 * c + 3
            pOp = psp.tile([128, 1024], F32, tag="pO", bufs=1, name="pOp")
            for j in range(jmax + 1):
                off = j * 128
                d = j - 4 * c
                nlo = max(0, 128 * d)
                qv = slice(c * 512 + nlo, (c + 1) * 512)
                pS = psp.tile([128, 1024], F32, tag="pS", bufs=2, name="pS")
                nc.tensor.matmul(pS[:, nlo:512],
                                 kp[p][0:64, off:off + 128],
                                 qp[p][0:64, qv],
                                 start=True, stop=True,
                                 tile_position=(0, 0))
                nc.tensor.matmul(pS[:, 512 + nlo:1024],
                                 kp[p][64:128, off:off + 128],
                                 qp[p][64:128, qv],
                                 start=True, stop=True,
                                 tile_position=(64, 0))
                pt = ptp.tile([128, 1024], F16, tag="pt", name="pt")
                if d < 0:
                    nc.scalar.activation(pt, pS, AF.Exp, scale=0.125)
                else:
                    pt_r = pt.rearrange("p (h w) -> p h w", w=512)[:, :, nlo:]
                    pS_r = pS.rearrange("p (h w) -> p h w", w=512)[:, :, nlo:]
                    nc.scalar.activation(pt_r, pS_r, AF.Exp, scale=0.125)
                    for hh in range(2):
                        blk = slice(512 * hh + nlo, 512 * hh + nlo + 128)
                        nc.gpsimd.tensor_mul(pt[:, blk], pt[:, blk], tri)
                for hh in range(2):
                    vsl = vp[:, j, 128 * (2 * p + hh):128 * (2 * p + hh + 1)]
                    nc.tensor.matmul(
                        pOp[:, 512 * hh + nlo:512 * (hh + 1)],
                        vsl,
                        pt[:, 512 * hh + nlo:512 * (hh + 1)],
                        start=(j == 0), stop=(j == jmax))
            # pO layout: A num rows 0-63 / den rows 64-127 (cols 0-511);
            # B den rows 0-63 / num rows 64-127 (cols 512-1023).
            # one fast copy evacuates pO so the next chunk's AV can start
            osb = rp2.tile([128, 1024], F32, tag="osb", name="osb")
            nc.vector.tensor_copy(osb, pOp)
            recb = rp2.tile([128, 1024], F32, tag="recb", bufs=2,
                            name="recb")
            nc.vector.reciprocal(recb[0:64, 0:512], osb[64:128, 0:512])
            nc.vector.reciprocal(recb[64:128, 512:1024],
                                 osb[0:64, 512:1024])
            nc.vector.tensor_mul(ypr[p][0:64, cs], osb[0:64, 0:512],
                                 recb[0:64, 0:512])
            nc.gpsimd.tensor_mul(ypr[p][64:128, cs],
                                 osb[64:128, 512:1024],
                                 recb[64:128, 512:1024])

        def emit_oproj_chunk(c, wide=False):
            cs = slice(c * 512, (c + 1) * 512)
            if wide:
                # post-attention: the pS banks are idle; use them in
                # dt-pairs for a deeper psum pipeline in the drain
                for dh in range(4):
                    po2 = psp.tile([128, 1024], F32, tag="pS", bufs=2,
                                   name="po2")
                    for half in range(2):
                        dt = 2 * dh + half
                        for kk in range(4):
                            nc.tensor.matmul(
                                po2[:, half * 512:(half + 1) * 512],
                                oc_fl[:, kk * 1024 + dt * 128:
                                      kk * 1024 + (dt + 1) * 128],
                                ypr[kk][:, cs],
                                start=(kk == 0), stop=(kk == 3))
                    osb3 = rp2.tile([128, 1024], F16, tag="osb3",
                                    name="osb3")
                    _copy(nc, True, osb3[:, 0:512], po2[:, 0:512])
                    _copy(nc, False, osb3[:, 512:1024], po2[:, 512:1024])
                    for half in range(2):
                        dt = 2 * dh + half
                        eng = (nc.gpsimd, nc.sync, nc.scalar)[(2 * dh + half)
                                                              % 3]
                        eng.dma_start(
                            out=OT[dt * 128:(dt + 1) * 128, cs],
                            in_=osb3[:, half * 512:(half + 1) * 512])
                return
            for dt in range(8):
                po = psp.tile([128, 512], F32, tag="ps", name="po")
                for kk in range(4):
                    nc.tensor.matmul(po,
                                     oc_fl[:, kk * 1024 + dt * 128:
                                           kk * 1024 + (dt + 1) * 128],
                                     ypr[kk][:, cs],
                                     start=(kk == 0), stop=(kk == 3))
                osb2 = rp2.tile([128, 512], F16, tag="os", bufs=3,
                                name="osb2")
                _copy(nc, (dt + c) % 2 == 0, osb2, po)
                nc.gpsimd.dma_start(out=OT[dt * 128:(dt + 1) * 128, cs],
                                    in_=osb2)

        # emission: pair-0 projections, then pair-outer attention with
        # next-pair projections and V tiles interleaved
        for c in range(NC_):
            emit_proj_chunk(wq_pair[0], 0, c, qp[0])
        for c in range(NC_):
            emit_proj_chunk(wk_pair[0], 0, c, kp[0])
        for j in range(4):
            emit_v_tile(j)
        for p in range(NP):
            for c in range(NC_):
                emit_attn_chunk(p, c)
                if p == 0 and c < 3:
                    for j in range(4 * c + 4, 4 * c + 8):
                        emit_v_tile(j)
                if p < 3:
                    if c == 0 and p + 2 < NP:
                        load_qk_pair(p + 2)
                    emit_proj_chunk(wq_pair[p + 1], p + 1, c, qp[p + 1])
                    emit_proj_chunk(wk_pair[p + 1], p + 1, c, kp[p + 1])
        # o_proj emitted last: lowest priority, so its matmuls fill PE
        # gaps while the final pair's exp-bound attention drains
        for c in range(NC_):
            emit_oproj_chunk(c, wide=(c == 3))

    nc.finalize()
    return nc


def _prep_inputs(x, q_proj, k_proj, v_proj, o_proj):
    f16 = np.float16
    pos = np.arange(S, dtype=np.float64)
    inv = THETA ** (-np.arange(0, DK, 2, dtype=np.float64) / DK)   # [32]
    ang = inv[:, None] * pos[None, :]                              # [32, S]
    cos32, sin32 = np.cos(ang), np.sin(ang)
    # interleaved rope rows: row 2i -> freq i; sin sign: + on even rows
    # (multiplies e_i for the odd output), - on odd rows
    cos64 = np.repeat(cos32, 2, axis=0)                            # [64, S]
    sin64 = np.repeat(sin32, 2, axis=0)
    sin64[1::2] *= -1.0
    cos_big = np.tile(cos64, (2, 1)).astype(f16)                   # [128, S]
    sin_big = np.tile(sin64, (2, 1)).astype(f16)

    in_maps = []
    for core in range(NCORES):
        b, g = core // 2, core % 2
        heads = [g * HPC + i for i in range(HPC)]
        nat = [h * DK + d_ for h in heads for d_ in range(DK)]

        def wlay(w):   # [D, HD] -> [128, NK*HD]
            return np.ascontiguousarray(
                w.reshape(NK, 128, HD).transpose(1, 0, 2)).astype(
                    f16).reshape(128, NK * HD)

        xt4 = (x[b].T.reshape(NK, 128, NC_, 512).transpose(1, 2, 0, 3)
               .reshape(128, NC_ * NK * 512))
        oc4 = (o_proj[:, nat].T.reshape(HD // 128, 128, D)
               .transpose(1, 0, 2).reshape(128, (HD // 128) * D))
        blob = np.concatenate([
            np.ascontiguousarray(xt4).astype(f16),
            wlay(q_proj[nat, :].T),
            wlay(k_proj[nat, :].T),
            wlay(v_proj[nat, :].T),
            np.ascontiguousarray(oc4).astype(f16),
            cos_big,
            sin_big,
        ], axis=1).reshape(128, 9, NK, 512)
        in_maps.append({"BLOB": blob})
    return in_maps


def _build_sharded(nc, n_cores=NCORES):
    """jit/shard_map callable mirroring run_bass_via_pjrt, built once."""
    install_neuronx_cc_hook()

    partition_name = (nc.partition_id_tensor.name
                      if nc.partition_id_tensor else None)
    in_names, out_names, out_avals, zero_outs = [], [], [], []
    for alloc in nc.m.functions[0].allocations:
        if not isinstance(alloc, mybir.MemoryLocationSet):
            continue
        name = alloc.memorylocations[0].name
        if alloc.kind == "ExternalInput":
            if name != partition_name:
                in_names.append(name)
        elif alloc.kind == "ExternalOutput":
            shape = tuple(alloc.tensor_shape)
            dtype = mybir.dt.np(alloc.dtype)
            out_names.append(name)
            out_avals.append(jax.core.ShapedArray(shape, dtype))
            zero_outs.append(np.zeros(shape, dtype))
    n_params = len(in_names)
    n_outs = len(out_avals)
    all_names = list(in_names) + list(out_names)
    if partition_name is not None:
        all_names.append(partition_name)

    donate = tuple(range(n_params, n_params + n_outs))

    def _body(*args):
        operands = list(args)
        if partition_name is not None:
            operands.append(partition_id_tensor())
        outs = _bass_exec_p.bind(
            *operands,
            out_avals=tuple(out_avals),
            in_names=tuple(all_names),
            out_names=tuple(out_names),
            lowering_input_output_aliases=(),
            sim_require_finite=True,
            sim_require_nnan=True,
            nc=nc,
        )
        return tuple(outs)

    devices = jax.devices()[:n_cores]
    mesh = Mesh(np.asarray(devices), ("core",))
    in_specs = (PartitionSpec("core"),) * (n_params + n_outs)
    out_specs = (PartitionSpec("core"),) * len(out_names)
    sharded = jax.jit(
        shard_map(
            _body, mesh=mesh, in_specs=in_specs, out_specs=out_specs,
            check_rep=False,
        ),
        donate_argnums=donate,
        keep_unused=True,
    )
    return {
        "sharded": sharded,
        "mesh": mesh,
        "in_names": in_names,
        "out_names": out_names,
        "out_avals": out_avals,
        "zero_outs": zero_outs,
        "n_cores": n_cores,
    }


def _get_runner():
    if "runner" not in _CACHE:
        if "nc" not in _CACHE:
            _CACHE["nc"] = _build_nc()
        _CACHE["runner"] = _build_sharded(_CACHE["nc"])
    return _CACHE["runner"]


def _run_in_maps(in_maps):
    """One full 8-core execution from host numpy in_maps -> per-core
    output dicts."""
    r = _get_runner()
    n_cores = r["n_cores"]
    concat_in = [
        np.concatenate([np.asarray(in_maps[c][name])
                        for c in range(n_cores)], axis=0)
        for name in r["in_names"]
    ]
    concat_zeros = [
        np.zeros((n_cores * z.shape[0], *z.shape[1:]), z.dtype)
        for z in r["zero_outs"]
    ]
    out_arrs = r["sharded"](*concat_in, *concat_zeros)
    return [
        {
            name: np.asarray(out_arrs[i]).reshape(
                n_cores, *r["out_avals"][i].shape)[c]
            for i, name in enumerate(r["out_names"])
        }
        for c in range(n_cores)
    ]


def kernel(x, q_proj, k_proj, v_proj, o_proj):
    x = np.asarray(x, dtype=np.float32)
    in_maps = _prep_inputs(x,
                           np.asarray(q_proj, dtype=np.float32),
                           np.asarray(k_proj, dtype=np.float32),
                           np.asarray(v_proj, dtype=np.float32),
                           np.asarray(o_proj, dtype=np.float32))
    results = _run_in_maps(in_maps)
    B = x.shape[0]
    out = np.empty((B, S, D), dtype=np.float32)
    for b in range(B):
        ot = (results[2 * b]["OT"].astype(np.float32)
              + results[2 * b + 1]["OT"].astype(np.float32))
        out[b] = ot.T
    return out
